# revision 30
# baseline (speedup 1.0000x reference)
"""Causal multi-head self-attention (RoPE) Trainium2 kernel.

Model (from the reference nn.Module):
  D_MODEL=1024, NUM_HEADS=16, D_K=64, THETA=10000, BATCH=2, SEQ=2048.
  qkv = x @ w_qkv.T ; q,k get interleaved-pair RoPE; causal softmax(q k^T/8) v;
  out = attn_out @ w_o.T.

Sharding: tensor-parallel over heads. 8 cores x 2 heads each. x is
replicated (transposed on host), per-core w_qkv/w_o head slices. Each core
produces a partial y.T (full [1024, 4096]); host sums partials and
transposes back.

Schedule: one merged software-pipelined stream. Attention segments
(b, qi) issue scores(kj) -> exp(kj) [ACT] with PV(kj-2) lagged two
iterations behind so the tensor engine never in-order-blocks on the exp.
QKV chunk production and the final w_o projection are "filler" work items
drained between attention iterations to keep PE busy during ACT-bound
stretches. Causal masking: one [128,128] triangle-mask matmul on the
first 128 columns of each diagonal block. Softmax denominators come from
an appended ones column in the PV stationary operand; normalization is
DVE reciprocal (from PSUM) + gpsimd partition_broadcast + DVE multiply.

PSUM: scores pool 2 tiles x 2 banks + ot pool 2 x 1 + filler 2 x 1 = 8.
"""

import math
from collections import deque
from functools import partial
import numpy as np
from contextlib import ExitStack

import concourse.bacc as bacc
import concourse.mybir as mybir
import concourse.tile as tile
from concourse.bass_utils import run_bass_kernel_spmd

f32 = mybir.dt.float32
f16 = mybir.dt.float16
bf16 = mybir.dt.bfloat16
f8e4 = mybir.dt.float8e4

D = 1024          # d_model
H = 16            # total heads
DK = 64           # head dim
B = 2
S = 2048
T = B * S         # 4096 tokens
NCORES = 8
HPC = H // NCORES  # heads per core = 2
THETA = 10000.0
NEG = -30000.0     # causal-mask additive constant (exp underflows to 0)

TCH = 512          # token chunk (matmul N)
NTCH = T // TCH    # 8
KCH = 128          # key chunk (score-tile partitions)
NBLK = T // KCH    # 32

SWAP_MASK = [m ^ 1 for m in range(32)]  # adjacent-pair swap, per 32-quadrant

_PROGRAM = None


def _build_program():
    nc = bacc.Bacc("TRN2", target_bir_lowering=False, debug=False)

    xT = nc.dram_tensor("xT", [D, T], f16, kind="ExternalInput")
    xT8 = nc.dram_tensor("xT8", [D, T], f8e4, kind="ExternalInput")
    wvT = nc.dram_tensor("wvT", [D, 128], f16, kind="ExternalInput")
    wqk8 = nc.dram_tensor("wqk8", [D, 256], f8e4, kind="ExternalInput")
    woT = nc.dram_tensor("woT", [128, D], f16, kind="ExternalInput")
    crep = nc.dram_tensor("crep", [128, S], f16, kind="ExternalInput")
    ssign = nc.dram_tensor("ssign", [128, S], f16, kind="ExternalInput")
    masktri = nc.dram_tensor("masktri", [128, 128], f16, kind="ExternalInput")
    identr = nc.dram_tensor("identr", [128, 128], f16, kind="ExternalInput")
    yT = nc.dram_tensor("yT", [D, T], bf16, kind="ExternalOutput")

    xT_r = xT.rearrange("(n p) t -> n p t", p=128)          # [8, 128, T]
    xT8_p = xT8.rearrange("(n p) t -> p n t", p=128)        # [128, 8, T]
    wv_r = wvT.rearrange("(n p) c -> p n c", p=128)         # [128, 8, 128]
    wqk8_r = wqk8.rearrange("(n p) c -> p n c", p=128)      # [128, 8, 256]

    with tile.TileContext(nc) as tc:
        with ExitStack() as ctx:
            singles = ctx.enter_context(tc.tile_pool(name="singles", bufs=1))

            wv_sb = singles.tile([128, 8, 128], f16)
            wqk8_sb = singles.tile([128, 8, 256], f8e4)
            crep_sb = singles.tile([128, S], f16)
            ssign_sb = singles.tile([128, S], f16)
            mask_sb = singles.tile([128, 128], f16)
            identr_sb = singles.tile([128, 128], f16)
            wo_sb = singles.tile([128, D], f16)

            qT = singles.tile([128, T], f16)
            kT = singles.tile([128, T], f16)
            # V in natural layout per 128-token block:
            # cols 0:64 = V_A, col 64 = ones, 65:129 = V_B, col 129 = ones.
            vaug = singles.tile([128, NBLK, 130], f16)
            nc.gpsimd.memset(vaug[:, :, 64], 1.0)
            nc.gpsimd.memset(vaug[:, :, 129], 1.0)
            ocatT = singles.tile([128, T], f16)

            xpool = ctx.enter_context(tc.tile_pool(name="xc", bufs=4))
            x8pool = ctx.enter_context(tc.tile_pool(name="xc8", bufs=4))
            ropep = ctx.enter_context(tc.tile_pool(name="rope", bufs=3))
            eps_p = ctx.enter_context(tc.tile_pool(name="e", bufs=6))
            nrm = ctx.enter_context(tc.tile_pool(name="nrm", bufs=4))
            yp = ctx.enter_context(tc.tile_pool(name="y", bufs=4))

            ps_fill = ctx.enter_context(
                tc.tile_pool(name="pf", bufs=2, space="PSUM"))   # 2 banks
            ps_s = ctx.enter_context(
                tc.tile_pool(name="ss", bufs=2, space="PSUM"))   # 4 banks
            ps_ot = ctx.enter_context(
                tc.tile_pool(name="ot", bufs=2, space="PSUM"))   # 2 banks

            xc_tiles = {}

            # ---------- filler work items (qkv chunks + projection) -------
            filler = deque()
            drained = [0]
            pending_rows = [0]
            iters_left = [2 * (4 + 8 + 12 + 16)]  # 80 attention iterations

            def push(rows, fn):
                filler.append((rows, fn))
                pending_rows[0] += rows

            def pop_one():
                rows, fn = filler.popleft()
                fn()
                drained[0] += 1
                pending_rows[0] -= rows
                return rows

            def drain_until(idx):
                while drained[0] <= idx:
                    pop_one()

            def drain_adaptive():
                # spread remaining filler evenly over remaining iterations
                budget = pending_rows[0] // max(iters_left[0], 1) + 128
                while filler and budget > 0:
                    budget -= pop_one()

            def drain_all():
                while filler:
                    pop_one()

            def emit_xdma(tch):
                # per-dc f16 DMAs (for V): matmul dc starts as its slice
                # lands; fp8 copy (for q/k) in pair-split DMAs
                xc = xpool.tile([128, 8, TCH], f16, tag="xc")
                xc8 = x8pool.tile([128, 8, TCH], f8e4, tag="xc8")
                t0 = tch * TCH
                for i in range(4):
                    nc.sync.dma_start(out=xc8[:, 2 * i:2 * i + 2, :],
                                      in_=xT8_p[:, 2 * i:2 * i + 2,
                                                t0:t0 + TCH])
                for dc in range(8):
                    nc.sync.dma_start(out=xc[:, dc, :],
                                      in_=xT_r[dc, :, t0:t0 + TCH])
                xc_tiles[tch] = (xc, xc8)

            def emit_qk(tch, mb):   # mb 0=q, 1=k
                _, xc8 = xc_tiles[tch]
                t0 = tch * TCH
                s0 = t0 % S  # RoPE tables repeat per batch
                ps = ps_fill.tile([128, TCH], f32, tag="pf")
                for i in range(4):  # fp8 DoubleRow: 2 d-chunks per matmul
                    nc.tensor.matmul(
                        ps, wqk8_sb[:, 2 * i:2 * i + 2,
                                    mb * 128:(mb + 1) * 128],
                        xc8[:, 2 * i:2 * i + 2, :],
                        start=(i == 0), stop=(i == 3),
                        perf_mode=mybir.MatmulPerfMode.DoubleRow)
                dst = qT if mb == 0 else kT
                sh = ropep.tile([128, TCH], f32, tag="sh")
                nc.vector.stream_shuffle(out=sh, in_=ps, mask=SWAP_MASK)
                tm1 = ropep.tile([128, TCH], f32, tag="tm1")
                nc.vector.tensor_tensor(
                    out=tm1, in0=ps, in1=crep_sb[:, s0:s0 + TCH],
                    op=mybir.AluOpType.mult)
                tm2 = ropep.tile([128, TCH], f32, tag="tm2")
                nc.vector.tensor_tensor(
                    out=tm2, in0=sh, in1=ssign_sb[:, s0:s0 + TCH],
                    op=mybir.AluOpType.mult)
                nc.vector.tensor_tensor(
                    out=dst[:, t0:t0 + TCH], in0=tm1, in1=tm2,
                    op=mybir.AluOpType.add)

            def emit_v(tch):
                xc, _ = xc_tiles[tch]
                pv = ps_fill.tile([128, 4, KCH], f32, tag="pf")
                for sub in range(4):
                    for dc in range(8):
                        nc.tensor.matmul(
                            pv[:, sub, :],
                            xc[:, dc, sub * KCH:(sub + 1) * KCH],
                            wv_sb[:, dc, :],
                            start=(dc == 0), stop=(dc == 7),
                            skip_group_check=True)
                b0 = tch * 4
                nc.vector.tensor_copy(
                    out=vaug[:, b0:b0 + 4, 0:64], in_=pv[:, :, 0:64])
                nc.scalar.activation(
                    out=vaug[:, b0:b0 + 4, 65:129], in_=pv[:, :, 64:128],
                    func=mybir.ActivationFunctionType.Copy)

            _ycnt = [0]

            def emit_proj(b, qi, eb):
                h0 = b * S + qi * TCH
                pys = ps_fill.tile([128, TCH], f32, tag="pf")
                nc.tensor.matmul(
                    pys, wo_sb[:, eb * 128:(eb + 1) * 128],
                    ocatT[:, h0:h0 + TCH], start=True, stop=True)
                y_sb = yp.tile([128, TCH], bf16, tag="ysb")
                if _ycnt[0] % 4 != 3:  # 3:1 DVE:ACT — keep ACT free for exp
                    nc.vector.tensor_copy(out=y_sb, in_=pys)
                else:
                    nc.scalar.activation(
                        out=y_sb, in_=pys,
                        func=mybir.ActivationFunctionType.Copy)
                _ycnt[0] += 1
                nc.sync.dma_start(
                    out=yT[eb * 128:(eb + 1) * 128, h0:h0 + TCH], in_=y_sb)

            # ---------- attention segment --------------------------------
            def attn_segment(b, qi):
                toff = b * S
                boff = b * (S // KCH)
                q0 = toff + qi * TCH
                nkj = 4 * qi + 4
                otA = ps_ot.tile([65, TCH], f32, tag="ot")
                otB = ps_ot.tile([65, TCH], f32, tag="ot")
                sc = {}
                ee = {}

                def scores(kj):
                    k0 = toff + kj * KCH
                    o = max(0, KCH * (kj - 4 * qi))
                    diag = kj >= 4 * qi
                    pAB = ps_s.tile([128, 2, TCH], f32, tag="sps")
                    nc.tensor.matmul(
                        pAB[:, 0, o:TCH], kT[0:64, k0:k0 + KCH],
                        qT[0:64, q0 + o:q0 + TCH],
                        start=True, stop=not diag, skip_group_check=True)
                    nc.tensor.matmul(
                        pAB[:, 1, o:TCH], kT[64:128, k0:k0 + KCH],
                        qT[64:128, q0 + o:q0 + TCH],
                        start=True, stop=not diag, skip_group_check=True)
                    if diag:  # triangle mask on first 128 cols only
                        nc.tensor.matmul(
                            pAB[:, 0, o:o + KCH], identr_sb, mask_sb,
                            start=False, stop=True, skip_group_check=True)
                        nc.tensor.matmul(
                            pAB[:, 1, o:o + KCH], identr_sb, mask_sb,
                            start=False, stop=True, skip_group_check=True)
                    sc[kj] = (pAB, o)

                def expf(kj):
                    pAB, o = sc.pop(kj)
                    eAB = eps_p.tile([128, 2, TCH], f16, tag="eT")
                    # wq,wk pre-scaled x8 on host: scores arrive x512
                    nc.scalar.activation(
                        out=eAB[:, :, o:TCH], in_=pAB[:, :, o:TCH],
                        func=mybir.ActivationFunctionType.Exp,
                        scale=1.0 / 512.0)
                    ee[kj] = (eAB, o)

                def pv(kj):
                    eAB, o = ee.pop(kj)
                    blk = boff + kj
                    nc.tensor.matmul(
                        otA[:, o:TCH], vaug[:, blk, 0:65], eAB[:, 0, o:TCH],
                        start=(kj == 0), stop=(kj == nkj - 1),
                        skip_group_check=True)
                    nc.tensor.matmul(
                        otB[:, o:TCH], vaug[:, blk, 65:130], eAB[:, 1, o:TCH],
                        start=(kj == 0), stop=(kj == nkj - 1),
                        skip_group_check=True)

                for kj in range(nkj):
                    scores(kj)
                    expf(kj)
                    if kj >= 2:
                        pv(kj - 2)
                    iters_left[0] -= 1
                    drain_adaptive()
                pv(nkj - 2)
                pv(nkj - 1)

                for hi, otX in ((0, otA), (1, otB)):
                    rX = nrm.tile([1, TCH], f32, tag="rr")
                    nc.vector.reciprocal(out=rX, in_=otX[64:65, :])
                    bcX = nrm.tile([64, TCH], f32, tag="bc")
                    nc.gpsimd.partition_broadcast(bcX, rX, channels=64)
                    nc.vector.tensor_tensor(
                        out=ocatT[hi * 64:(hi + 1) * 64, q0:q0 + TCH],
                        in0=otX[0:64, :], in1=bcX,
                        op=mybir.AluOpType.mult)

            # ---------- emission -----------------------------------------
            # Prologue DMA order: q-weights + first x chunk feed the first
            # matmuls; RoPE tables land before the first rope; the rest
            # follows. All on the HWDGE (sync) queue.
            nc.sync.dma_start(out=wqk8_sb[:, :, 0:128],
                              in_=wqk8_r[:, :, 0:128])
            nc.sync.dma_start(out=wqk8_sb[:, :, 128:256],
                              in_=wqk8_r[:, :, 128:256])
            emit_xdma(0)
            nc.sync.dma_start(out=crep_sb, in_=crep[:, :])
            nc.sync.dma_start(out=ssign_sb, in_=ssign[:, :])
            nc.sync.dma_start(out=wv_sb, in_=wv_r[:, :, :])
            emit_xdma(1)
            emit_xdma(2)
            nc.sync.dma_start(out=mask_sb, in_=masktri[:, :])
            nc.sync.dma_start(out=identr_sb, in_=identr[:, :])
            nc.sync.dma_start(out=wo_sb, in_=woT[:, :])
            emit_xdma(3)

            chunk_last = {}
            for tch in range(NTCH):
                if tch + 4 < NTCH:
                    push(0, partial(emit_xdma, tch + 4))
                push(1024, partial(emit_qk, tch, 0))
                push(1024, partial(emit_qk, tch, 1))
                push(4096, partial(emit_v, tch))
                chunk_last[tch] = len(filler) - 1

            # prologue: chunks 0 and 1 fully emitted before attention
            drain_until(chunk_last[1])

            # batch 0 ascending (attention starts early); batch 1 descending
            # (the long 16-iter segment gets the chunk-drain filler, short
            # segments at the end are fed by accumulated proj items)
            for b, qi in ((0, 0), (0, 1), (0, 2), (0, 3),
                          (1, 3), (1, 2), (1, 1), (1, 0)):
                drain_until(chunk_last[b * 4 + qi])
                attn_segment(b, qi)
                for eb in range(8):  # proj for this qi ready now
                    push(512, partial(emit_proj, b, qi, eb))
            drain_all()

    nc.compile()
    return nc


def _host_prep(x, token_positions, w_qkv, w_o):
    """Build per-core input maps."""
    x = np.asarray(x, dtype=np.float32)
    w_qkv = np.asarray(w_qkv, dtype=np.float32)
    w_o = np.asarray(w_o, dtype=np.float32)
    pos = np.asarray(token_positions).astype(np.float64)

    xT = np.ascontiguousarray(x.reshape(T, D).T).astype(np.float16)

    half = DK // 2
    inv_freq = THETA ** (-np.arange(half, dtype=np.float64) / half)  # [32]
    ang = pos[:, None] * inv_freq[None, :]          # [S, 32]
    cos = np.cos(ang).astype(np.float16)            # [S, 32]
    sin = np.sin(ang).astype(np.float16)

    # interleaved pair layout: partition p (within a head's 64) has freq p//2
    cos_rows = np.repeat(cos.T, 2, axis=0)          # [64, S]
    sin_rows = np.repeat(sin.T, 2, axis=0)
    sgn = np.where(np.arange(64) % 2 == 0, -1.0, 1.0).astype(np.float16)
    ssin_rows = sin_rows * sgn[:, None]
    crep = np.vstack([cos_rows, cos_rows])          # [128, 2048]
    ssign = np.vstack([ssin_rows, ssin_rows])

    # triangle mask: col j of a diagonal 128-block is masked for key p > j
    jj = np.arange(128)[None, :]
    pp = np.arange(128)[:, None]
    masktri = np.where(jj >= pp, 0.0, NEG).astype(np.float16)

    identr_np = np.eye(128, dtype=np.float16)

    import ml_dtypes
    f8 = ml_dtypes.float8_e4m3
    xT8 = xT.astype(np.float32).astype(f8)

    # q/k weights x8 into fp8 (exp applies 1/512 = 1/(8*8*sqrt(64)));
    # v/o weights stay f16
    in_maps = []
    for c in range(NCORES):
        hA, hB = HPC * c, HPC * c + 1
        wqk = np.empty((256, D), dtype=np.float32)
        wqk[0:64] = w_qkv[hA * DK:(hA + 1) * DK] * 8.0
        wqk[64:128] = w_qkv[hB * DK:(hB + 1) * DK] * 8.0
        wqk[128:192] = w_qkv[D + hA * DK:D + (hA + 1) * DK] * 8.0
        wqk[192:256] = w_qkv[D + hB * DK:D + (hB + 1) * DK] * 8.0
        wqk8 = np.ascontiguousarray(wqk.T).astype(f8)

        wv = np.empty((128, D), dtype=np.float32)
        wv[0:64] = w_qkv[2 * D + hA * DK:2 * D + (hA + 1) * DK]
        wv[64:128] = w_qkv[2 * D + hB * DK:2 * D + (hB + 1) * DK]
        wvT = np.ascontiguousarray(wv.T).astype(np.float16)

        woTc = np.ascontiguousarray(
            w_o[:, hA * DK:(hB + 1) * DK].T).astype(np.float16)  # [128, 1024]

        in_maps.append({
            "xT": xT, "xT8": xT8, "wvT": wvT, "wqk8": wqk8, "woT": woTc,
            "crep": crep, "ssign": ssign, "masktri": masktri,
            "identr": identr_np,
        })
    return in_maps


def _get_program():
    global _PROGRAM
    if _PROGRAM is None:
        _PROGRAM = _build_program()
    return _PROGRAM


def run_sharded(in_maps, **kwargs):
    nc = _get_program()
    return run_bass_kernel_spmd(nc, in_maps, core_ids=list(range(NCORES)),
                                **kwargs)


def kernel(x, token_positions, w_qkv, w_o):
    in_maps = _host_prep(x, token_positions, w_qkv, w_o)
    res = run_sharded(in_maps)
    acc = np.zeros((D, T), dtype=np.float64)
    for c in range(NCORES):
        acc += res.results[c]["yT"].astype(np.float32)
    y = acc.T.astype(np.float32).reshape(B, S, D)
    return y


# revision 36
# speedup vs baseline: 1.0533x; 1.0533x over previous
"""Causal multi-head self-attention (RoPE) Trainium2 kernel.

Model (from the reference nn.Module):
  D_MODEL=1024, NUM_HEADS=16, D_K=64, THETA=10000, BATCH=2, SEQ=2048.
  qkv = x @ w_qkv.T ; q,k get interleaved-pair RoPE; causal softmax(q k^T/8) v;
  out = attn_out @ w_o.T.

Sharding: tensor-parallel over heads. 8 cores x 2 heads each. x is
replicated (transposed on host), per-core w_qkv/w_o head slices. Each core
produces a partial y.T (full [1024, 4096]); host sums partials and
transposes back.

Schedule: one merged software-pipelined stream. Attention segments
(b, qi) issue scores(kj) -> exp(kj) [ACT] with PV(kj-2) lagged two
iterations behind so the tensor engine never in-order-blocks on the exp.
QKV chunk production and the final w_o projection are "filler" work items
drained between attention iterations to keep PE busy during ACT-bound
stretches. Causal masking: one [128,128] triangle-mask matmul on the
first 128 columns of each diagonal block. Softmax denominators come from
an appended ones column in the PV stationary operand; normalization is
DVE reciprocal (from PSUM) + gpsimd partition_broadcast + DVE multiply.

PSUM: scores pool 2 tiles x 2 banks + ot pool 2 x 1 + filler 2 x 1 = 8.
"""

import math
from collections import deque
from functools import partial
import numpy as np
from contextlib import ExitStack

import concourse.bacc as bacc
import concourse.mybir as mybir
import concourse.tile as tile
from concourse.bass_utils import run_bass_kernel_spmd

f32 = mybir.dt.float32
f16 = mybir.dt.float16
bf16 = mybir.dt.bfloat16
f8e4 = mybir.dt.float8e4

D = 1024          # d_model
H = 16            # total heads
DK = 64           # head dim
B = 2
S = 2048
T = B * S         # 4096 tokens
NCORES = 8
HPC = H // NCORES  # heads per core = 2
THETA = 10000.0
NEG = -30000.0     # causal-mask additive constant (exp underflows to 0)

TCH = 512          # token chunk (matmul N)
NTCH = T // TCH    # 8
KCH = 128          # key chunk (score-tile partitions)
NBLK = T // KCH    # 32

SWAP_MASK = [m ^ 1 for m in range(32)]  # adjacent-pair swap, per 32-quadrant

_PROGRAM = None


def _build_program():
    nc = bacc.Bacc("TRN2", target_bir_lowering=False, debug=False)

    xT = nc.dram_tensor("xT", [D, T], f16, kind="ExternalInput")
    xT8 = nc.dram_tensor("xT8", [D, T], f8e4, kind="ExternalInput")
    wvT = nc.dram_tensor("wvT", [D, 128], f16, kind="ExternalInput")
    wqk8 = nc.dram_tensor("wqk8", [D, 256], f8e4, kind="ExternalInput")
    woT = nc.dram_tensor("woT", [128, D], f16, kind="ExternalInput")
    crep = nc.dram_tensor("crep", [128, S], f16, kind="ExternalInput")
    ssign = nc.dram_tensor("ssign", [128, S], f16, kind="ExternalInput")
    masktri = nc.dram_tensor("masktri", [128, 128], f16, kind="ExternalInput")
    identr = nc.dram_tensor("identr", [128, 128], f16, kind="ExternalInput")
    yT = nc.dram_tensor("yT", [D, T], bf16, kind="ExternalOutput")

    xT_r = xT.rearrange("(n p) t -> n p t", p=128)          # [8, 128, T]
    xT_p = xT.rearrange("(n p) t -> p n t", p=128)          # [128, 8, T]
    xT8_p = xT8.rearrange("(n p) t -> p n t", p=128)        # [128, 8, T]
    wv_r = wvT.rearrange("(n p) c -> p n c", p=128)         # [128, 8, 128]
    wqk8_r = wqk8.rearrange("(n p) c -> p n c", p=128)      # [128, 8, 256]
    yT_p = yT.rearrange("(e p) t -> p e t", p=128)          # [128, 8, T]

    with tile.TileContext(nc) as tc:
        with ExitStack() as ctx:
            singles = ctx.enter_context(tc.tile_pool(name="singles", bufs=1))

            wv_sb = singles.tile([128, 8, 128], f16)
            wqk8_sb = singles.tile([128, 8, 256], f8e4)
            crep_sb = singles.tile([128, S], f16)
            ssign_sb = singles.tile([128, S], f16)
            mask_sb = singles.tile([128, 128], f16)
            identr_sb = singles.tile([128, 128], f16)
            wo_sb = singles.tile([128, D], f16)

            qT = singles.tile([128, T], f16)
            kT = singles.tile([128, T], f16)
            # V in natural layout per 128-token block:
            # cols 0:64 = V_A, col 64 = ones, 65:129 = V_B, col 129 = ones.
            vaug = singles.tile([128, NBLK, 130], f16)
            nc.gpsimd.memset(vaug[:, :, 64], 1.0)
            nc.gpsimd.memset(vaug[:, :, 129], 1.0)
            ocatT = singles.tile([128, T], f16)

            xpool = ctx.enter_context(tc.tile_pool(name="xc", bufs=4))
            x8pool = ctx.enter_context(tc.tile_pool(name="xc8", bufs=4))
            ropep = ctx.enter_context(tc.tile_pool(name="rope", bufs=3))
            eps_p = ctx.enter_context(tc.tile_pool(name="e", bufs=6))
            nrm = ctx.enter_context(tc.tile_pool(name="nrm", bufs=4))
            yp = ctx.enter_context(tc.tile_pool(name="y", bufs=4))

            ps_fill = ctx.enter_context(
                tc.tile_pool(name="pf", bufs=2, space="PSUM"))   # 2 banks
            ps_s = ctx.enter_context(
                tc.tile_pool(name="ss", bufs=2, space="PSUM"))   # 4 banks
            ps_ot = ctx.enter_context(
                tc.tile_pool(name="ot", bufs=2, space="PSUM"))   # 2 banks

            xc_tiles = {}

            # ---------- filler work items (qkv chunks + projection) -------
            filler = deque()
            drained = [0]
            pending_rows = [0]
            iters_left = [2 * (4 + 8 + 12 + 16)]  # 80 attention iterations

            def push(rows, fn):
                filler.append((rows, fn))
                pending_rows[0] += rows

            def pop_one():
                rows, fn = filler.popleft()
                fn()
                drained[0] += 1
                pending_rows[0] -= rows
                return rows

            def drain_until(idx):
                while drained[0] <= idx:
                    pop_one()

            def drain_adaptive():
                # spread remaining filler evenly over remaining iterations
                budget = pending_rows[0] // max(iters_left[0], 1) + 128
                while filler and budget > 0:
                    budget -= pop_one()

            def drain_all():
                while filler:
                    pop_one()

            def emit_xdma(tch, split=False):
                xc = xpool.tile([128, 8, TCH], f16, tag="xc")
                xc8 = x8pool.tile([128, 8, TCH], f8e4, tag="xc8")
                t0 = tch * TCH
                if split:  # prologue chunk: fine-grained so compute starts
                    for i in range(4):
                        nc.sync.dma_start(out=xc8[:, 2 * i:2 * i + 2, :],
                                          in_=xT8_p[:, 2 * i:2 * i + 2,
                                                    t0:t0 + TCH])
                    for dc in range(8):
                        nc.sync.dma_start(out=xc[:, dc, :],
                                          in_=xT_r[dc, :, t0:t0 + TCH])
                else:
                    nc.sync.dma_start(out=xc8, in_=xT8_p[:, :, t0:t0 + TCH])
                    nc.sync.dma_start(out=xc, in_=xT_p[:, :, t0:t0 + TCH])
                xc_tiles[tch] = (xc, xc8)

            def emit_qk(tch, mb):   # mb 0=q, 1=k
                _, xc8 = xc_tiles[tch]
                t0 = tch * TCH
                s0 = t0 % S  # RoPE tables repeat per batch
                ps = ps_fill.tile([128, TCH], f32, tag="pf")
                for i in range(4):  # fp8 DoubleRow: 2 d-chunks per matmul
                    nc.tensor.matmul(
                        ps, wqk8_sb[:, 2 * i:2 * i + 2,
                                    mb * 128:(mb + 1) * 128],
                        xc8[:, 2 * i:2 * i + 2, :],
                        start=(i == 0), stop=(i == 3),
                        perf_mode=mybir.MatmulPerfMode.DoubleRow)
                dst = qT if mb == 0 else kT
                # f16 intermediates: the all-SBUF all-16-bit ops get DVE
                # 2x/4x perf modes
                sh = ropep.tile([128, TCH], f16, tag="sh")
                nc.vector.stream_shuffle(out=sh, in_=ps, mask=SWAP_MASK)
                tm1 = ropep.tile([128, TCH], f16, tag="tm1")
                nc.vector.tensor_tensor(
                    out=tm1, in0=ps, in1=crep_sb[:, s0:s0 + TCH],
                    op=mybir.AluOpType.mult)
                tm2 = ropep.tile([128, TCH], f16, tag="tm2")
                nc.vector.tensor_tensor(
                    out=tm2, in0=sh, in1=ssign_sb[:, s0:s0 + TCH],
                    op=mybir.AluOpType.mult)
                nc.vector.tensor_tensor(
                    out=dst[:, t0:t0 + TCH], in0=tm1, in1=tm2,
                    op=mybir.AluOpType.add)

            def emit_v(tch):
                xc, _ = xc_tiles[tch]
                pv = ps_fill.tile([128, 4, KCH], f32, tag="pf")
                for sub in range(4):
                    for dc in range(8):
                        nc.tensor.matmul(
                            pv[:, sub, :],
                            xc[:, dc, sub * KCH:(sub + 1) * KCH],
                            wv_sb[:, dc, :],
                            start=(dc == 0), stop=(dc == 7),
                            skip_group_check=True)
                b0 = tch * 4
                nc.vector.tensor_copy(
                    out=vaug[:, b0:b0 + 4, 0:64], in_=pv[:, :, 0:64])
                nc.scalar.activation(
                    out=vaug[:, b0:b0 + 4, 65:129], in_=pv[:, :, 64:128],
                    func=mybir.ActivationFunctionType.Copy)

            _ycnt = [0]
            _ytiles = {}

            def emit_proj(b, qi, eb):
                h0 = b * S + qi * TCH
                pys = ps_fill.tile([128, TCH], f32, tag="pf")
                nc.tensor.matmul(
                    pys, wo_sb[:, eb * 128:(eb + 1) * 128],
                    ocatT[:, h0:h0 + TCH], start=True, stop=True)
                if (b, qi) not in _ytiles:
                    y_sb = yp.tile([128, 8, TCH], bf16, tag="ysb",
                                   name=f"ysb_{b}_{qi}")
                    _ytiles[(b, qi)] = y_sb
                y_sb = _ytiles[(b, qi)]
                if _ycnt[0] % 4 != 3:  # 3:1 DVE:ACT — keep ACT free for exp
                    nc.vector.tensor_copy(out=y_sb[:, eb, :], in_=pys)
                else:
                    nc.scalar.activation(
                        out=y_sb[:, eb, :], in_=pys,
                        func=mybir.ActivationFunctionType.Copy)
                _ycnt[0] += 1
                if eb == 7:  # all embedding blocks staged: one DMA per qi
                    nc.sync.dma_start(out=yT_p[:, :, h0:h0 + TCH], in_=y_sb)
                    del _ytiles[(b, qi)]

            # ---------- attention segment --------------------------------
            def attn_segment(b, qi):
                toff = b * S
                boff = b * (S // KCH)
                q0 = toff + qi * TCH
                nkj = 4 * qi + 4
                otA = ps_ot.tile([65, TCH], f32, tag="ot")
                otB = ps_ot.tile([65, TCH], f32, tag="ot")
                sc = {}
                ee = {}

                def scores(kj):
                    k0 = toff + kj * KCH
                    o = max(0, KCH * (kj - 4 * qi))
                    diag = kj >= 4 * qi
                    pAB = ps_s.tile([128, 2, TCH], f32, tag="sps")
                    nc.tensor.matmul(
                        pAB[:, 0, o:TCH], kT[0:64, k0:k0 + KCH],
                        qT[0:64, q0 + o:q0 + TCH],
                        start=True, stop=not diag, skip_group_check=True)
                    nc.tensor.matmul(
                        pAB[:, 1, o:TCH], kT[64:128, k0:k0 + KCH],
                        qT[64:128, q0 + o:q0 + TCH],
                        start=True, stop=not diag, skip_group_check=True)
                    if diag:  # triangle mask on first 128 cols only
                        nc.tensor.matmul(
                            pAB[:, 0, o:o + KCH], identr_sb, mask_sb,
                            start=False, stop=True, skip_group_check=True)
                        nc.tensor.matmul(
                            pAB[:, 1, o:o + KCH], identr_sb, mask_sb,
                            start=False, stop=True, skip_group_check=True)
                    sc[kj] = (pAB, o)

                def expf(kj):
                    pAB, o = sc.pop(kj)
                    eAB = eps_p.tile([128, 2, TCH], f16, tag="eT")
                    # wq,wk pre-scaled x8 on host: scores arrive x512
                    nc.scalar.activation(
                        out=eAB[:, :, o:TCH], in_=pAB[:, :, o:TCH],
                        func=mybir.ActivationFunctionType.Exp,
                        scale=1.0 / 512.0)
                    ee[kj] = (eAB, o)

                def pv(kj):
                    eAB, o = ee.pop(kj)
                    blk = boff + kj
                    nc.tensor.matmul(
                        otA[:, o:TCH], vaug[:, blk, 0:65], eAB[:, 0, o:TCH],
                        start=(kj == 0), stop=(kj == nkj - 1),
                        skip_group_check=True)
                    nc.tensor.matmul(
                        otB[:, o:TCH], vaug[:, blk, 65:130], eAB[:, 1, o:TCH],
                        start=(kj == 0), stop=(kj == nkj - 1),
                        skip_group_check=True)

                for kj in range(nkj):
                    scores(kj)
                    expf(kj)
                    if kj >= 2:
                        pv(kj - 2)
                    iters_left[0] -= 1
                    drain_adaptive()
                pv(nkj - 2)
                pv(nkj - 1)

                for hi, otX in ((0, otA), (1, otB)):
                    rX = nrm.tile([1, TCH], f32, tag="rr")
                    nc.vector.reciprocal(out=rX, in_=otX[64:65, :])
                    bcX = nrm.tile([64, TCH], f32, tag="bc")
                    nc.gpsimd.partition_broadcast(bcX, rX, channels=64)
                    nc.vector.tensor_tensor(
                        out=ocatT[hi * 64:(hi + 1) * 64, q0:q0 + TCH],
                        in0=otX[0:64, :], in1=bcX,
                        op=mybir.AluOpType.mult)

            # ---------- emission -----------------------------------------
            # Prologue DMA order: q-weights + first x chunk feed the first
            # matmuls; RoPE tables land before the first rope; the rest
            # follows. All on the HWDGE (sync) queue.
            nc.sync.dma_start(out=wqk8_sb[:, :, 0:128],
                              in_=wqk8_r[:, :, 0:128])
            nc.sync.dma_start(out=wqk8_sb[:, :, 128:256],
                              in_=wqk8_r[:, :, 128:256])
            emit_xdma(0, split=True)
            nc.sync.dma_start(out=crep_sb, in_=crep[:, :])
            nc.sync.dma_start(out=ssign_sb, in_=ssign[:, :])
            nc.sync.dma_start(out=wv_sb, in_=wv_r[:, :, :])
            emit_xdma(1)
            emit_xdma(2)
            nc.sync.dma_start(out=mask_sb, in_=masktri[:, :])
            nc.sync.dma_start(out=identr_sb, in_=identr[:, :])
            nc.sync.dma_start(out=wo_sb, in_=woT[:, :])
            emit_xdma(3)

            chunk_last = {}
            for tch in range(NTCH):
                if tch + 4 < NTCH:
                    push(0, partial(emit_xdma, tch + 4))
                push(1024, partial(emit_qk, tch, 0))
                push(1024, partial(emit_qk, tch, 1))
                push(4096, partial(emit_v, tch))
                chunk_last[tch] = len(filler) - 1

            # prologue: chunks 0 and 1 fully emitted before attention
            drain_until(chunk_last[1])

            # batch 0 ascending (attention starts early); batch 1 descending
            # (the long 16-iter segment gets the chunk-drain filler, short
            # segments at the end are fed by accumulated proj items)
            for b, qi in ((0, 0), (0, 1), (0, 2), (0, 3),
                          (1, 3), (1, 2), (1, 1), (1, 0)):
                drain_until(chunk_last[b * 4 + qi])
                attn_segment(b, qi)
                for eb in range(8):  # proj for this qi ready now
                    push(512, partial(emit_proj, b, qi, eb))
            drain_all()

    nc.compile()
    return nc


def _host_prep(x, token_positions, w_qkv, w_o):
    """Build per-core input maps."""
    x = np.asarray(x, dtype=np.float32)
    w_qkv = np.asarray(w_qkv, dtype=np.float32)
    w_o = np.asarray(w_o, dtype=np.float32)
    pos = np.asarray(token_positions).astype(np.float64)

    xT = np.ascontiguousarray(x.reshape(T, D).T).astype(np.float16)

    half = DK // 2
    inv_freq = THETA ** (-np.arange(half, dtype=np.float64) / half)  # [32]
    ang = pos[:, None] * inv_freq[None, :]          # [S, 32]
    cos = np.cos(ang).astype(np.float16)            # [S, 32]
    sin = np.sin(ang).astype(np.float16)

    # interleaved pair layout: partition p (within a head's 64) has freq p//2
    cos_rows = np.repeat(cos.T, 2, axis=0)          # [64, S]
    sin_rows = np.repeat(sin.T, 2, axis=0)
    sgn = np.where(np.arange(64) % 2 == 0, -1.0, 1.0).astype(np.float16)
    ssin_rows = sin_rows * sgn[:, None]
    crep = np.vstack([cos_rows, cos_rows])          # [128, 2048]
    ssign = np.vstack([ssin_rows, ssin_rows])

    # triangle mask: col j of a diagonal 128-block is masked for key p > j
    jj = np.arange(128)[None, :]
    pp = np.arange(128)[:, None]
    masktri = np.where(jj >= pp, 0.0, NEG).astype(np.float16)

    identr_np = np.eye(128, dtype=np.float16)

    import ml_dtypes
    f8 = ml_dtypes.float8_e4m3
    xT8 = xT.astype(np.float32).astype(f8)

    # q/k weights x8 into fp8 (exp applies 1/512 = 1/(8*8*sqrt(64)));
    # v/o weights stay f16
    in_maps = []
    for c in range(NCORES):
        hA, hB = HPC * c, HPC * c + 1
        wqk = np.empty((256, D), dtype=np.float32)
        wqk[0:64] = w_qkv[hA * DK:(hA + 1) * DK] * 8.0
        wqk[64:128] = w_qkv[hB * DK:(hB + 1) * DK] * 8.0
        wqk[128:192] = w_qkv[D + hA * DK:D + (hA + 1) * DK] * 8.0
        wqk[192:256] = w_qkv[D + hB * DK:D + (hB + 1) * DK] * 8.0
        wqk8 = np.ascontiguousarray(wqk.T).astype(f8)

        wv = np.empty((128, D), dtype=np.float32)
        wv[0:64] = w_qkv[2 * D + hA * DK:2 * D + (hA + 1) * DK]
        wv[64:128] = w_qkv[2 * D + hB * DK:2 * D + (hB + 1) * DK]
        wvT = np.ascontiguousarray(wv.T).astype(np.float16)

        woTc = np.ascontiguousarray(
            w_o[:, hA * DK:(hB + 1) * DK].T).astype(np.float16)  # [128, 1024]

        in_maps.append({
            "xT": xT, "xT8": xT8, "wvT": wvT, "wqk8": wqk8, "woT": woTc,
            "crep": crep, "ssign": ssign, "masktri": masktri,
            "identr": identr_np,
        })
    return in_maps


def _get_program():
    global _PROGRAM
    if _PROGRAM is None:
        _PROGRAM = _build_program()
    return _PROGRAM


def run_sharded(in_maps, **kwargs):
    nc = _get_program()
    return run_bass_kernel_spmd(nc, in_maps, core_ids=list(range(NCORES)),
                                **kwargs)


def kernel(x, token_positions, w_qkv, w_o):
    in_maps = _host_prep(x, token_positions, w_qkv, w_o)
    res = run_sharded(in_maps)
    acc = np.zeros((D, T), dtype=np.float64)
    for c in range(NCORES):
        acc += res.results[c]["yT"].astype(np.float32)
    y = acc.T.astype(np.float32).reshape(B, S, D)
    return y
